# revision 6
# baseline (speedup 1.0000x reference)
"""Trainium2 Bass kernel for de-emphasis IIR: y[n] = x[n] + 0.97*y[n-1] along last axis.

Input: waveform (32, 2, 480000) f32 = 64 independent sequences of 480k samples.
Sharding: pure data parallel — 8 sequences per core across 8 NeuronCores.

Per core: the 8 sequences are split into 16 chunks each -> 128 partitions,
each owning a contiguous 30000-sample chunk. The recurrence y = c*y_prev + x
runs along the free dim with the hardware DVE scan (tensor_tensor_scan).
Chunk boundaries are handled with an H-sample halo warmup (influence decays
as 0.97^H; H=720 -> 3e-10, far below fp32 noise), so partitions are fully
independent and no cross-partition or cross-core communication is needed.
"""

import numpy as np

COEFF = 0.97

# Full-problem geometry (hardcoded; harness runs kernel() standalone).
N_CORES = 8
SEQ_TOTAL = 64  # 32*2
S = SEQ_TOTAL // N_CORES  # 8 sequences per core
N = 480000  # samples per sequence
K = 16  # chunks per sequence -> S*K = 128 partitions
H = 720  # halo (warmup) samples per chunk
W = 5120  # scan/DMA tile width (free dim)

_BUILD_CACHE = {}


def build_deemph(S, N, K, H, W, coeff=COEFF, bufs=3):
    """Build the Bass program for one core: x[S,N] -> y[S,N]."""
    import concourse.bacc as bacc
    import concourse.mybir as mybir
    from concourse.mybir import AluOpType
    from concourse.tile import TileContext

    C = N // K  # chunk length
    P = S * K  # partitions
    assert N % K == 0, (N, K)
    assert (C + H) % W == 0, (C, H, W)
    assert W > H
    T = (C + H) // W  # tiles per chunk
    f32 = mybir.dt.float32

    nc = bacc.Bacc(trn_type="TRN2", debug=False)
    x = nc.dram_tensor("x", [S, N], f32, kind="ExternalInput")
    y = nc.dram_tensor("y", [S, N], f32, kind="ExternalOutput")
    # [K, S, C] views: DMA pairing maps (k, s) -> partition k*S + s
    xt = x[:].rearrange("s (k j) -> s k j", k=K).transpose((1, 0, 2))
    yt = y[:].rearrange("s (k j) -> s k j", k=K).transpose((1, 0, 2))

    with TileContext(nc) as tc:
        with (
            tc.tile_pool(name="cpool", bufs=1) as cpool,
            tc.tile_pool(name="xpool", bufs=bufs) as xpool,
            tc.tile_pool(name="ypool", bufs=bufs) as ypool,
        ):
            ctile = cpool.tile([P, 1], f32)
            nc.vector.memset(ctile[:, :], coeff)
            cbcast = ctile[:, 0:1].broadcast_to((P, W))
            prev_y = None
            for i in range(T):
                xtile = xpool.tile([P, W], f32, tag="xt")
                if i == 0:
                    # chunk 0 of each seq (partitions 0..S): zero warmup
                    nc.vector.memset(xtile[0:S, 0:H], 0.0)
                    nc.sync.dma_start(xtile[0:S, H:W], x[:, 0 : W - H])
                    # chunks k>=1: halo = tail of chunk k-1, then head of chunk k
                    nc.sync.dma_start(xtile[S:P, 0:H], xt[0 : K - 1, :, C - H : C])
                    nc.sync.dma_start(xtile[S:P, H:W], xt[1:K, :, 0 : W - H])
                else:
                    lo = W * i - H
                    nc.sync.dma_start(xtile[:, :], xt[:, :, lo : lo + W])
                ytile = ypool.tile([P, W], f32, tag="yt")
                init = 0.0 if i == 0 else prev_y[:, W - 1 : W]
                nc.vector.tensor_tensor_scan(
                    ytile[:, :],
                    cbcast,
                    xtile[:, :],
                    init,
                    AluOpType.mult,
                    AluOpType.add,
                )
                if i == 0:
                    nc.scalar.dma_start(yt[:, :, 0 : W - H], ytile[:, H:W])
                else:
                    lo = W * i - H
                    nc.scalar.dma_start(yt[:, :, lo : lo + W], ytile[:, :])
                prev_y = ytile
    nc.compile()
    return nc


def _get_nc():
    key = (S, N, K, H, W)
    if key not in _BUILD_CACHE:
        _BUILD_CACHE[key] = build_deemph(*key)
    return _BUILD_CACHE[key]


def run(waveform: np.ndarray, **spmd_kwargs):
    """Run on 8 NeuronCores; returns (full_output, BassKernelResults)."""
    from concourse.bass_utils import run_bass_kernel_spmd

    waveform = np.asarray(waveform)
    orig_shape = waveform.shape
    x = np.ascontiguousarray(waveform.reshape(SEQ_TOTAL, N).astype(np.float32, copy=False))
    nc = _get_nc()
    in_maps = [{"x": x[S * c : S * (c + 1)]} for c in range(N_CORES)]
    res = run_bass_kernel_spmd(nc, in_maps, core_ids=list(range(N_CORES)), **spmd_kwargs)
    out = np.concatenate([r["y"] for r in res.results], axis=0)
    return out.reshape(orig_shape), res


def kernel(waveform: np.ndarray) -> np.ndarray:
    out, _ = run(waveform)
    return out
